# revision 8
# baseline (speedup 1.0000x reference)
"""RWKV-4 block (2 att channels + ReLU^2 FFN) on 8 Trainium2 NeuronCores.

Sharding: data-parallel over batch B=8 -> one batch element per core.
Each core runs the full block for its batch element; no collectives.

Layout strategy per core:
  - LayerNorms run in [T,C] layout (free-dim reduction), then activations are
    transposed on the PE to [C,T] so every matmul contracts over the partition
    dim. Weights are pre-transposed/pre-cast to bf16 on the host.
  - The WKV recurrence is restructured as: (1) a max-plus scan for pp via
    tensor_tensor_scan(add,max), (2) elementwise exp coefficients, (3) two
    linear scans for aa/bb via tensor_tensor_scan(mult,add). This reproduces
    the reference's stabilization path exactly; scans chunk over T with state
    chained through small per-feature state tiles.
"""
from contextlib import ExitStack

import numpy as np
import ml_dtypes

import concourse.bass as bass
import concourse.bacc as bacc
import concourse.mybir as mybir
import concourse.tile as tile
from concourse import masks
from concourse.bass_utils import run_bass_kernel_spmd

F32 = mybir.dt.float32
BF16 = mybir.dt.bfloat16
AX = mybir.AxisListType
OP = mybir.AluOpType
AF = mybir.ActivationFunctionType
bf16 = ml_dtypes.bfloat16

A, B, T, C, D, F = 2, 8, 2048, 1024, 1024, 4096
SHIFT = 2
EPS = 1e-5
NCORES = 8
TCH = 512              # T chunk (matmul N / scan chunk)
NT = T // 128          # 16 t-tiles
NCT = C // 128         # 8 c-tiles
ND = D // 128          # 8 d-chunks
NF = F // 128          # 32 f-tiles
NFH = NF // 2          # f-tiles per half
NN = T // TCH          # 4 T-chunks

_PROGRAM_CACHE = {}


def build_program():
    if "nc" in _PROGRAM_CACHE:
        return _PROGRAM_CACHE["nc"]
    nc = bacc.Bacc("TRN2", target_bir_lowering=False, debug=False,
                   num_devices=NCORES)

    io = {}
    def din(name, shape, dt):
        io[name] = nc.dram_tensor(name, shape, dt, kind="ExternalInput")
    def dout(name, shape, dt):
        io[name] = nc.dram_tensor(name, shape, dt, kind="ExternalOutput")

    din("x_in", [T, C], F32)
    din("ln_consts", [4, 128, C], F32)
    din("td_all", [128, A * ND], F32)
    din("u_all", [128, A * ND], F32)
    din("st0", [3, 128, A * ND], F32)
    din("mix_att", [128, 96], F32)
    din("mix_ffn", [128, 32], F32)
    din("wkT", [A, C, D], BF16)
    din("wvT", [A, C, D], BF16)
    din("wrT", [A, C, D], BF16)
    din("woT", [A, D, C], BF16)
    din("shiftT", [A, C, SHIFT], BF16)
    din("fwkT", [C, F], BF16)
    din("fwvT", [F, C], BF16)
    din("fwrT", [C, C], BF16)
    din("fshiftT", [C, 1], BF16)

    dout("out_x", [T, C], F32)
    dout("out_xln1", [T, C], F32)
    dout("out_state", [3, 128, A * ND], F32)
    dout("out_ln2last", [1, C], F32)

    io["att_sc"] = [nc.dram_tensor(f"att_sc{a}", [C, T], F32) for a in range(A)]
    io["x2_sc"] = nc.dram_tensor("x2_sc", [T, C], F32)

    with tile.TileContext(nc) as tc:
        _build(tc, io)
    nc.compile()
    _PROGRAM_CACHE["nc"] = nc
    return nc


def _build(tc, io):
    nc = tc.nc
    octx = ExitStack()

    # ---------------- persistent constants ----------------
    cpool = octx.enter_context(tc.tile_pool(name="consts", bufs=1))
    ident = cpool.tile([128, 128], F32, name="ident")
    masks.make_identity(nc, ident[:])
    def load_lnw(pool, i):
        t_ = pool.tile([128, C], F32, name=f"lnw{i}", tag=f"lnw{i}", bufs=1)
        nc.sync.dma_start(out=t_[:], in_=io["ln_consts"][i])
        return t_

    w_all = cpool.tile([128, A * ND], F32, name="w_all")
    nc.sync.dma_start(out=w_all[:], in_=io["td_all"][:, :])
    nc.scalar.activation(w_all[:], w_all[:], AF.Exp)
    nc.vector.tensor_scalar_mul(w_all[:], w_all[:], -1.0)
    u_all = cpool.tile([128, A * ND], F32, name="u_all")
    nc.sync.dma_start(out=u_all[:], in_=io["u_all"][:, :])
    stS = []
    for j in range(3):
        t_ = cpool.tile([128, A * ND], F32, name=f"stS{j}")
        nc.sync.dma_start(out=t_[:], in_=io["st0"][j])
        stS.append(t_)
    mixA = cpool.tile([128, 96], F32, name="mixA")
    nc.sync.dma_start(out=mixA[:], in_=io["mix_att"][:, :])
    mixF = cpool.tile([128, 32], F32, name="mixF")
    nc.sync.dma_start(out=mixF[:], in_=io["mix_ffn"][:, :])

    # ---------------- helpers ----------------
    def layernorm(pool, x_t, w_t, b_t, tag):
        s_ = pool.tile([128, 1], F32, name=f"{tag}_s", tag=f"{tag}_s")
        nc.vector.reduce_sum(s_[:], x_t[:], axis=AX.X)
        m_ = pool.tile([128, 1], F32, name=f"{tag}_m", tag=f"{tag}_m")
        nc.vector.tensor_scalar_mul(m_[:], s_[:], 1.0 / C)
        nc.vector.tensor_scalar(x_t[:], x_t[:], m_[:], None, OP.subtract)
        sq = pool.tile([128, C], F32, name=f"{tag}_sq", tag=f"{tag}_sq")
        ss = pool.tile([128, 1], F32, name=f"{tag}_ss", tag=f"{tag}_ss")
        nc.scalar.activation(sq[:], x_t[:], AF.Square, accum_out=ss[:])
        v_ = pool.tile([128, 1], F32, name=f"{tag}_v", tag=f"{tag}_v")
        nc.vector.tensor_scalar(v_[:], ss[:], 1.0 / C, EPS, OP.mult, OP.add)
        nc.scalar.activation(v_[:], v_[:], AF.Sqrt)
        r_ = pool.tile([128, 1], F32, name=f"{tag}_r", tag=f"{tag}_r")
        nc.vector.reciprocal(r_[:], v_[:])
        o_ = pool.tile([128, C], F32, name=f"{tag}_o", tag=f"{tag}_o")
        nc.vector.scalar_tensor_tensor(o_[:], x_t[:], r_[:], w_t[:], OP.mult, OP.mult)
        nc.gpsimd.tensor_add(o_[:], o_[:], b_t[:])
        return o_

    def transpose_cast(pspool, src_view, dst_view):
        ps = pspool.tile([128, 128], F32, name="tr_ps", tag="tr_ps")
        nc.tensor.transpose(ps[:], src_view, ident[:])
        nc.scalar.copy(dst_view, ps[:])

    # ============ PHASES 1-3 under a scope that frees XT afterwards ============
    with ExitStack() as attctx:
        xt_pool = attctx.enter_context(tc.tile_pool(name="xt", bufs=1))
        XT = [xt_pool.tile([128, T], BF16, name=f"xtr{j}", tag=f"xtr{j}")
              for j in range(NCT)]
        # ---- PHASE 1: LN1 + transpose ----
        with tc.tile_pool(name="p1", bufs=2) as p1, \
             tc.tile_pool(name="p1ps", bufs=2, space="PSUM") as p1ps:
            lnw0, lnw1 = load_lnw(p1, 0), load_lnw(p1, 1)
            for tt in range(NT):
                xtile = p1.tile([128, C], F32, name="xtile", tag="xtile")
                nc.sync.dma_start(out=xtile[:], in_=io["x_in"][tt * 128:(tt + 1) * 128, :])
                xln = layernorm(p1, xtile, lnw0, lnw1, "ln1")
                nc.sync.dma_start(out=io["out_xln1"][tt * 128:(tt + 1) * 128, :],
                                  in_=xln[:])
                for j in range(NCT):
                    transpose_cast(p1ps, xln[:, j * 128:(j + 1) * 128],
                                   XT[j][:, tt * 128:(tt + 1) * 128])

        # ---- PHASES 2+3: per-channel time-mix ----
        for a in range(A):
            with ExitStack() as actx:
                wp = actx.enter_context(tc.tile_pool(name=f"wp{a}", bufs=1))
                w_sb = [[], [], []]
                for i, src in enumerate((io["wkT"], io["wvT"], io["wrT"])):
                    for ct in range(NCT):
                        t_ = wp.tile([128, D], BF16, name=f"wA{i}_{ct}",
                                     tag=f"wA{i}_{ct}")
                        nc.sync.dma_start(out=t_[:],
                                          in_=src[a, ct * 128:(ct + 1) * 128, :])
                        w_sb[i].append(t_)
                wo_sb = []
                for dc in range(ND):
                    t_ = wp.tile([128, C], BF16, name=f"wo_{dc}", tag=f"wo_{dc}")
                    nc.sync.dma_start(out=t_[:],
                                      in_=io["woT"][a, dc * 128:(dc + 1) * 128, :])
                    wo_sb.append(t_)

                mp = actx.enter_context(tc.tile_pool(name=f"mp{a}", bufs=1))
                sp = actx.enter_context(tc.tile_pool(name=f"sp{a}", bufs=2))
                syp = actx.enter_context(tc.tile_pool(name=f"syp{a}", bufs=2))
                obp = actx.enter_context(tc.tile_pool(name=f"obp{a}", bufs=1))
                psp = actx.enter_context(
                    tc.tile_pool(name=f"psp{a}", bufs=1, space="PSUM"))
                pso = actx.enter_context(
                    tc.tile_pool(name=f"pso{a}", bufs=2, space="PSUM"))

                xxh = []
                for ct in range(NCT):
                    t_ = mp.tile([128, TCH], BF16, name=f"xxh{ct}", tag=f"xxh{ct}")
                    nc.sync.dma_start(out=t_[:, 0:SHIFT],
                                      in_=io["shiftT"][a, ct * 128:(ct + 1) * 128, :])
                    nc.vector.tensor_copy(t_[:, SHIFT:TCH], XT[ct][:, 0:TCH - SHIFT])
                    xxh.append(t_)

                for n in range(NN):
                    n0 = n * TCH
                    rhs = [[None] * NCT for _ in range(3)]
                    for ct in range(NCT):
                        xxv = (xxh[ct][:, 0:TCH] if n == 0
                               else XT[ct][:, n0 - SHIFT:n0 + TCH - SHIFT])
                        for p in range(3):
                            mcol = a * 24 + p * 8 + ct
                            t1 = mp.tile([128, TCH], BF16, name="t1", tag="t1",
                                         bufs=2)
                            nc.vector.tensor_scalar_mul(
                                t1[:], XT[ct][:, n0:n0 + TCH],
                                mixA[:, mcol:mcol + 1])
                            r_ = mp.tile([128, TCH], BF16, name=f"rhs{p}_{ct}",
                                         tag=f"rhs{p}_{ct}")
                            nc.vector.scalar_tensor_tensor(
                                r_[:], xxv, mixA[:, 48 + mcol:48 + mcol + 1],
                                t1[:], OP.mult, OP.add)
                            rhs[p][ct] = r_

                    sry_n = [None] * ND
                    for dc in range(ND):
                        col = a * ND + dc
                        dsl = slice(dc * 128, (dc + 1) * 128)
                        psK = psp.tile([128, TCH], F32, name="psK", tag="psK")
                        psV = psp.tile([128, TCH], F32, name="psV", tag="psV")
                        psR = psp.tile([128, TCH], F32, name="psR", tag="psR")
                        for ct in range(NCT):
                            nc.tensor.matmul(psK[:], w_sb[0][ct][:, dsl],
                                             rhs[0][ct][:], start=(ct == 0),
                                             stop=(ct == NCT - 1))
                        for ct in range(NCT):
                            nc.tensor.matmul(psV[:], w_sb[1][ct][:, dsl],
                                             rhs[1][ct][:], start=(ct == 0),
                                             stop=(ct == NCT - 1))
                        for ct in range(NCT):
                            nc.tensor.matmul(psR[:], w_sb[2][ct][:, dsl],
                                             rhs[2][ct][:], start=(ct == 0),
                                             stop=(ct == NCT - 1))
                        k_sb = sp.tile([128, TCH], F32, name="k_sb", tag="k_sb")
                        nc.scalar.copy(k_sb[:], psK[:])
                        v_sb = sp.tile([128, TCH], F32, name="v_sb", tag="v_sb")
                        nc.scalar.copy(v_sb[:], psV[:])
                        sr_sb = sp.tile([128, TCH], BF16, name="sr_sb", tag="sr_sb")
                        nc.scalar.activation(sr_sb[:], psR[:], AF.Sigmoid)

                        # ---- WKV scan chunk ----
                        wcol = w_all[:, col:col + 1]
                        ucol = u_all[:, col:col + 1]
                        pp_init = stS[2][:, col:col + 1]
                        aa_init = stS[0][:, col:col + 1]
                        bb_init = stS[1][:, col:col + 1]
                        wbc = sp.tile([128, TCH], F32, name="wbc", tag="wbc")
                        nc.scalar.activation(wbc[:], k_sb[:], AF.Identity,
                                             bias=wcol, scale=0.0)
                        ppb = sp.tile([128, TCH + 1], F32, name="ppb", tag="ppb")
                        aab = sp.tile([128, TCH + 1], F32, name="aab", tag="aab")
                        bbb = sp.tile([128, TCH + 1], F32, name="bbb", tag="bbb")
                        nc.vector.tensor_copy(ppb[:, 0:1], pp_init)
                        nc.vector.tensor_copy(aab[:, 0:1], aa_init)
                        nc.vector.tensor_copy(bbb[:, 0:1], bb_init)
                        nc.vector.tensor_tensor_scan(
                            ppb[:, 1:TCH + 1], wbc[:], k_sb[:], pp_init,
                            OP.add, OP.max)
                        ppp = ppb[:, 0:TCH]
                        ppc = ppb[:, 1:TCH + 1]
                        tA = sp.tile([128, TCH], F32, name="tA", tag="tA")
                        nc.vector.tensor_sub(tA[:], ppp, ppc)
                        nc.scalar.activation(tA[:], tA[:], AF.Exp, bias=wcol)  # e1b
                        tB = sp.tile([128, TCH], F32, name="tB", tag="tB")
                        nc.vector.tensor_sub(tB[:], k_sb[:], ppc)
                        nc.scalar.activation(tB[:], tB[:], AF.Exp)             # e2b
                        tC = sp.tile([128, TCH], F32, name="tC", tag="tC")
                        nc.gpsimd.tensor_mul(tC[:], tB[:], v_sb[:])
                        nc.vector.tensor_tensor_scan(
                            aab[:, 1:TCH + 1], tA[:], tC[:], aa_init,
                            OP.mult, OP.add)
                        nc.vector.tensor_tensor_scan(
                            bbb[:, 1:TCH + 1], tA[:], tB[:], bb_init,
                            OP.mult, OP.add)
                        # chain state for next chunk
                        nc.vector.tensor_copy(pp_init, ppb[:, TCH:TCH + 1])
                        nc.vector.tensor_copy(aa_init, aab[:, TCH:TCH + 1])
                        nc.vector.tensor_copy(bb_init, bbb[:, TCH:TCH + 1])
                        # uk = k + u (in place)
                        nc.scalar.activation(k_sb[:], k_sb[:], AF.Identity,
                                             bias=ucol)
                        tD = sp.tile([128, TCH], F32, name="tD", tag="tD")
                        nc.vector.tensor_max(tD[:], ppp, k_sb[:])              # q
                        nc.gpsimd.tensor_sub(tA[:], ppp, tD[:])
                        nc.scalar.activation(tA[:], tA[:], AF.Exp)             # e1
                        nc.vector.tensor_sub(tB[:], k_sb[:], tD[:])
                        nc.scalar.activation(tB[:], tB[:], AF.Exp)             # e2
                        nc.vector.tensor_mul(tC[:], tA[:], aab[:, 0:TCH])
                        nc.gpsimd.tensor_mul(tD[:], tB[:], v_sb[:])
                        nc.vector.tensor_add(tC[:], tC[:], tD[:])              # num
                        nc.gpsimd.tensor_mul(tA[:], tA[:], bbb[:, 0:TCH])
                        nc.vector.tensor_add(tA[:], tA[:], tB[:])              # den
                        nc.vector.reciprocal(tA[:], tA[:])
                        nc.vector.tensor_mul(tC[:], tC[:], tA[:])              # y
                        sry = syp.tile([128, TCH], BF16, name=f"sry{dc}",
                                       tag=f"sry{dc}")
                        nc.vector.tensor_mul(sry[:], sr_sb[:], tC[:])
                        sry_n[dc] = sry

                    for ct in range(NCT):
                        psO = pso.tile([128, TCH], F32, name="psO", tag="psO")
                        for dc in range(ND):
                            nc.tensor.matmul(
                                psO[:], wo_sb[dc][:, ct * 128:(ct + 1) * 128],
                                sry_n[dc][:], start=(dc == 0), stop=(dc == ND - 1))
                        ob = obp.tile([128, TCH], F32, name="ob", tag="ob")
                        nc.scalar.copy(ob[:], psO[:])
                        nc.sync.dma_start(
                            out=io["att_sc"][a][ct * 128:(ct + 1) * 128,
                                                n0:n0 + TCH],
                            in_=ob[:])

    for j in range(3):
        nc.sync.dma_start(out=io["out_state"][j], in_=stS[j][:])

    # ============ PHASE 4: residual + LN2 + transpose ============
    xt2_pool = octx.enter_context(tc.tile_pool(name="xt2", bufs=1))
    XT2 = [xt2_pool.tile([128, T], BF16, name=f"xt2r{j}", tag=f"xt2r{j}")
           for j in range(NCT)]
    with tc.tile_pool(name="p4", bufs=2) as p4, \
         tc.tile_pool(name="p4ps", bufs=2, space="PSUM") as p4ps:
        lnw2, lnw3 = load_lnw(p4, 2), load_lnw(p4, 3)
        for tt in range(NT):
            tsl = slice(tt * 128, (tt + 1) * 128)
            xo = p4.tile([128, C], F32, name="xo", tag="xo")
            nc.sync.dma_start(out=xo[:], in_=io["x_in"][tsl, :])
            for ct in range(NCT):
                ab0 = p4.tile([128, 128], F32, name="ab0", tag="ab0")
                nc.sync.dma_start(out=ab0[:],
                                  in_=io["att_sc"][0][ct * 128:(ct + 1) * 128, tsl])
                ab1 = p4.tile([128, 128], F32, name="ab1", tag="ab1")
                nc.sync.dma_start(out=ab1[:],
                                  in_=io["att_sc"][1][ct * 128:(ct + 1) * 128, tsl])
                nc.vector.tensor_add(ab0[:], ab0[:], ab1[:])
                ps = p4ps.tile([128, 128], F32, name="p4tr", tag="p4tr")
                nc.tensor.transpose(ps[:], ab0[:], ident[:])
                nc.vector.tensor_add(xo[:, ct * 128:(ct + 1) * 128],
                                     xo[:, ct * 128:(ct + 1) * 128], ps[:])
            nc.sync.dma_start(out=io["x2_sc"][tsl, :], in_=xo[:])
            xln2 = layernorm(p4, xo, lnw2, lnw3, "ln2")
            if tt == NT - 1:
                nc.sync.dma_start(out=io["out_ln2last"][0:1, :],
                                  in_=xln2[127:128, :])
            for j in range(NCT):
                transpose_cast(p4ps, xln2[:, j * 128:(j + 1) * 128],
                               XT2[j][:, tt * 128:(tt + 1) * 128])

    # ============ PHASE 5: channel-mix FFN ============
    with ExitStack() as fctx:
        fwp = fctx.enter_context(tc.tile_pool(name="fwp", bufs=1))
        fwr_sb = []
        for ct in range(NCT):
            t_ = fwp.tile([128, C], BF16, name=f"fwr{ct}", tag=f"fwr{ct}")
            nc.sync.dma_start(out=t_[:], in_=io["fwrT"][ct * 128:(ct + 1) * 128, :])
            fwr_sb.append(t_)
        fkp = fctx.enter_context(tc.tile_pool(name="fkp", bufs=1))
        fvp = fctx.enter_context(tc.tile_pool(name="fvp", bufs=1))
        fmp = fctx.enter_context(tc.tile_pool(name="fmp", bufs=2))
        kfp = fctx.enter_context(tc.tile_pool(name="kfp", bufs=1))
        fsp = fctx.enter_context(tc.tile_pool(name="fsp", bufs=2))
        psf = fctx.enter_context(tc.tile_pool(name="psf", bufs=2, space="PSUM"))
        psg = fctx.enter_context(tc.tile_pool(name="psg", bufs=1, space="PSUM"))
        psv2 = fctx.enter_context(tc.tile_pool(name="psv2", bufs=2, space="PSUM"))
        pst5 = fctx.enter_context(tc.tile_pool(name="pst5", bufs=2, space="PSUM"))

        fxxh = []
        for ct in range(NCT):
            t_ = fmp.tile([128, TCH], BF16, name=f"fxxh{ct}", tag=f"fxxh{ct}",
                          bufs=1)
            nc.sync.dma_start(out=t_[:, 0:1],
                              in_=io["fshiftT"][ct * 128:(ct + 1) * 128, :])
            nc.vector.tensor_copy(t_[:, 1:TCH], XT2[ct][:, 0:TCH - 1])
            fxxh.append(t_)

        for n in range(NN):
            n0 = n * TCH
            fxk, fxr = [], []
            for ct in range(NCT):
                xxv = (fxxh[ct][:, 0:TCH] if n == 0
                       else XT2[ct][:, n0 - 1:n0 + TCH - 1])
                for p, lst in ((0, fxk), (1, fxr)):
                    mcol = p * 8 + ct
                    t1 = fmp.tile([128, TCH], BF16, name="ft1", tag="ft1")
                    nc.vector.tensor_scalar_mul(
                        t1[:], XT2[ct][:, n0:n0 + TCH], mixF[:, mcol:mcol + 1])
                    r_ = fmp.tile([128, TCH], BF16, name=f"fx{p}_{ct}",
                                  tag=f"fx{p}_{ct}", bufs=1)
                    nc.vector.scalar_tensor_tensor(
                        r_[:], xxv, mixF[:, 16 + mcol:16 + mcol + 1], t1[:],
                        OP.mult, OP.add)
                    lst.append(r_)
            # r gate
            srf = []
            for ct in range(NCT):
                ps = psg.tile([128, TCH], F32, name="psG", tag="psG")
                for c2 in range(NCT):
                    nc.tensor.matmul(ps[:], fwr_sb[c2][:, ct * 128:(ct + 1) * 128],
                                     fxr[c2][:], start=(c2 == 0),
                                     stop=(c2 == NCT - 1))
                sg = fsp.tile([128, TCH], F32, name=f"srf{ct}", tag=f"srf{ct}",
                              bufs=1)
                nc.scalar.activation(sg[:], ps[:], AF.Sigmoid)
                srf.append(sg)
            # kf + kv in two F halves (stream fwk/fwv halves, SBUF-accumulate kv)
            kvacc = []
            for ct in range(NCT):
                t_ = fsp.tile([128, TCH], F32, name=f"kva{ct}", tag=f"kva{ct}",
                              bufs=1)
                kvacc.append(t_)
            for h in range(2):
                fk_h = []
                for ct in range(NCT):
                    t_ = fkp.tile([128, F // 2], BF16, name=f"fkh{ct}",
                                  tag=f"fkh{ct}")
                    nc.sync.dma_start(
                        out=t_[:],
                        in_=io["fwkT"][ct * 128:(ct + 1) * 128,
                                       h * (F // 2):(h + 1) * (F // 2)])
                    fk_h.append(t_)
                kf_h = []
                for i in range(NFH):
                    ps = psf.tile([128, TCH], F32, name="psF", tag="psF")
                    for ct in range(NCT):
                        nc.tensor.matmul(ps[:], fk_h[ct][:, i * 128:(i + 1) * 128],
                                         fxk[ct][:], start=(ct == 0),
                                         stop=(ct == NCT - 1))
                    rt = fsp.tile([128, TCH], F32, name="rt", tag="rt")
                    nc.scalar.activation(rt[:], ps[:], AF.Relu)
                    kt = kfp.tile([128, TCH], BF16, name=f"kf{i}", tag=f"kf{i}")
                    nc.vector.tensor_mul(kt[:], rt[:], rt[:])
                    kf_h.append(kt)
                fv_h = []
                for i in range(NFH):
                    fc = h * NFH + i
                    t_ = fvp.tile([128, C], BF16, name=f"fv{i}", tag=f"fv{i}")
                    nc.sync.dma_start(out=t_[:],
                                      in_=io["fwvT"][fc * 128:(fc + 1) * 128, :])
                    fv_h.append(t_)
                for ct in range(NCT):
                    ps = psv2.tile([128, TCH], F32, name="psV2", tag="psV2")
                    for i in range(NFH):
                        nc.tensor.matmul(ps[:], fv_h[i][:, ct * 128:(ct + 1) * 128],
                                         kf_h[i][:], start=(i == 0),
                                         stop=(i == NFH - 1))
                    if h == 0:
                        nc.scalar.copy(kvacc[ct][:], ps[:])
                    else:
                        nc.vector.tensor_add(kvacc[ct][:], kvacc[ct][:], ps[:])
            # gate (in place into kvacc) + transpose + residual + store
            for ct in range(NCT):
                nc.vector.tensor_mul(kvacc[ct][:], srf[ct][:], kvacc[ct][:])
            for tl in range(TCH // 128):
                tsl = slice(n0 + tl * 128, n0 + (tl + 1) * 128)
                xo2 = fsp.tile([128, C], F32, name="xo2", tag="xo2")
                nc.sync.dma_start(out=xo2[:], in_=io["x2_sc"][tsl, :])
                for ct in range(NCT):
                    ps = pst5.tile([128, 128], F32, name="p5tr", tag="p5tr")
                    nc.tensor.transpose(ps[:], kvacc[ct][:, tl * 128:(tl + 1) * 128],
                                        ident[:])
                    nc.vector.tensor_add(xo2[:, ct * 128:(ct + 1) * 128],
                                         xo2[:, ct * 128:(ct + 1) * 128], ps[:])
                nc.sync.dma_start(out=io["out_x"][tsl, :], in_=xo2[:])

    octx.close()


# ---------------- host-side marshal / unmarshal ----------------

def _prep_shared(inputs):
    f32 = np.float32
    g = {}
    ln = np.stack([
        np.broadcast_to(inputs["ln1_w"].astype(f32), (128, C)),
        np.broadcast_to(inputs["ln1_b"].astype(f32), (128, C)),
        np.broadcast_to(inputs["ln2_w"].astype(f32), (128, C)),
        np.broadcast_to(inputs["ln2_b"].astype(f32), (128, C))])
    g["ln_consts"] = np.ascontiguousarray(ln)
    g["td_all"] = np.ascontiguousarray(
        inputs["att_time_decay"].astype(f32).reshape(A, ND, 128)
        .transpose(2, 0, 1).reshape(128, A * ND))
    g["u_all"] = np.ascontiguousarray(
        inputs["att_time_first"].astype(f32).reshape(A, ND, 128)
        .transpose(2, 0, 1).reshape(128, A * ND))
    mix = np.stack([inputs["att_mix_k"].reshape(A, C),
                    inputs["att_mix_v"].reshape(A, C),
                    inputs["att_mix_r"].reshape(A, C)], axis=1).astype(f32)
    m_t = mix.reshape(A, 3, NCT, 128).transpose(3, 0, 1, 2).reshape(128, 48)
    g["mix_att"] = np.ascontiguousarray(np.concatenate([m_t, 1.0 - m_t], axis=1))
    fmix = np.stack([inputs["ffn_mix_k"].reshape(C),
                     inputs["ffn_mix_r"].reshape(C)], axis=0).astype(f32)
    fm_t = fmix.reshape(2, NCT, 128).transpose(2, 0, 1).reshape(128, 16)
    g["mix_ffn"] = np.ascontiguousarray(np.concatenate([fm_t, 1.0 - fm_t], axis=1))
    g["wkT"] = np.ascontiguousarray(
        inputs["att_Wk"].astype(f32).transpose(0, 2, 1)).astype(bf16)
    g["wvT"] = np.ascontiguousarray(
        inputs["att_Wv"].astype(f32).transpose(0, 2, 1)).astype(bf16)
    g["wrT"] = np.ascontiguousarray(
        inputs["att_Wr"].astype(f32).transpose(0, 2, 1)).astype(bf16)
    g["woT"] = np.ascontiguousarray(
        inputs["att_Wo"].astype(f32).transpose(0, 2, 1)).astype(bf16)
    g["fwkT"] = np.ascontiguousarray(inputs["ffn_Wk"].astype(f32).T).astype(bf16)
    g["fwvT"] = np.ascontiguousarray(inputs["ffn_Wv"].astype(f32).T).astype(bf16)
    g["fwrT"] = np.ascontiguousarray(inputs["ffn_Wr"].astype(f32).T).astype(bf16)
    return g


def _prep_core(inputs, b):
    f32 = np.float32
    m = {}
    m["x_in"] = np.ascontiguousarray(inputs["x"][b].astype(f32))
    m["st0"] = np.ascontiguousarray(
        inputs["att_wkv_state"][:, b].astype(f32)
        .reshape(A, ND, 128, 3).transpose(3, 2, 0, 1).reshape(3, 128, A * ND))
    m["shiftT"] = np.ascontiguousarray(
        inputs["att_shift_state"][:, b].astype(f32)
        .transpose(0, 2, 1)).astype(bf16)
    m["fshiftT"] = np.ascontiguousarray(
        inputs["ffn_shift_state"][b].astype(f32).T).astype(bf16)
    return m


def run_on_device(inputs, **kw):
    nc = build_program()
    shared = _prep_shared(inputs)
    in_maps = [dict(shared, **_prep_core(inputs, b)) for b in range(NCORES)]
    res = run_bass_kernel_spmd(nc, in_maps, core_ids=list(range(NCORES)), **kw)
    return res


def assemble_outputs(inputs, results):
    f32 = np.float32
    x_out = np.stack([results[b]["out_x"] for b in range(B)]).astype(f32)
    xln1 = np.stack([results[b]["out_xln1"] for b in range(B)]).astype(f32)
    new_wkv = np.zeros((A, B, D, 3), f32)
    for b in range(B):
        st = results[b]["out_state"]
        for a in range(A):
            sub = st[:, :, a * ND:(a + 1) * ND]          # [3, 128, ND]
            new_wkv[a, b] = sub.transpose(2, 1, 0).reshape(D, 3)
    ln2_last = np.stack([results[b]["out_ln2last"][0] for b in range(B)]).astype(f32)
    xxx = np.concatenate(
        [inputs["att_shift_state"].astype(f32),
         np.broadcast_to(xln1[None], (A, B, T, C))], axis=2)
    return x_out, xxx, new_wkv, ln2_last


def kernel(**inputs):
    inputs = {k: np.asarray(v) for k, v in inputs.items()}
    res = run_on_device(inputs)
    return assemble_outputs(inputs, res.results)
